# revision 18
# baseline (speedup 1.0000x reference)
"""Trainium2 Bass kernel for nn_Attention_13537736917778.

Full inputs -> full output. Sharding: 8 cores = 2 (batch) x 4 (head groups of 4).
Per-core: channel-major flash attention (S^T layout, keys on partitions).
V is produced directly token-major (x-chunk stationaries) so no PE transposes
or PSUM->SBUF copy chains; softmax denominators come replicated from
ones-columns in the PV stationary and are inverted on DVE
(reciprocal_approx_fast), keeping the ACT engine exp-only during attention.
QKV/RMS for the second head pair and the out-projection are interleaved into
the attention emission in small atomic chunks so the PE queue never drains.
Out-projection partial sums are reduced on host.
"""
import sys
import numpy as np

sys.path.insert(0, "/opt/trn_rl_repo")

import ml_dtypes
import concourse.bass as bass
import concourse.mybir as mybir
from concourse import tile
from concourse.bass_utils import run_bass_kernel_spmd
from contextlib import ExitStack

bf16 = mybir.dt.bfloat16
f32 = mybir.dt.float32
f8 = mybir.dt.float8e4

B, N, C = 2, 2048, 1024
H, D = 16, 64
G = 4              # heads per core
NT = N             # tokens per core (one batch)
FT = 512
TI = NT // FT      # 4 i-tiles
KC = C // 128      # 8 input-channel chunks
JC = NT // 128     # 16 key chunks
EPS = 1e-6
SCALE = 1.0 / 8.0  # 1/sqrt(64)

_CACHE = {}


def _split_waits(nc, limit=1):
    """walrus CTRL has one hw wait slot; split multi-wait instructions into
    NOP chains carrying the extra waits."""
    counter = 0
    for fn in nc.m.functions:
        for bb in fn.blocks:
            new_insts = []
            for inst in bb.instructions:
                si = inst.sync_info
                if si is not None and si.on_wait and len(si.on_wait) > limit:
                    waits = list(si.on_wait)
                    head, tail = waits[:-limit], waits[-limit:]
                    for i in range(0, len(head), limit):
                        nop = mybir.InstNoOp(
                            name=f"I-waitsplit-{counter}", ins=[], outs=[]
                        )
                        counter += 1
                        nop.engine = inst.engine
                        nop.sync_info = mybir.SyncInfo(
                            on_wait=head[i : i + limit], on_update=[]
                        )
                        new_insts.append(nop)
                    inst.sync_info = mybir.SyncInfo(
                        on_wait=tail, on_update=list(si.on_update)
                    )
                new_insts.append(inst)
            bb.instructions[:] = new_insts
    return counter


def _dedupe_ldweights(nc):
    """Drop an InstLdweights identical to the previous one on the PE stream
    (only Matmult/NoOp between) -- the PE keeps the loaded weights."""
    pe = mybir.EngineType.PE
    ndel = 0
    for fn in nc.m.functions:
        for bb in fn.blocks:
            new = []
            last = None
            for inst in bb.instructions:
                tn = type(inst).__name__
                if getattr(inst, "engine", None) == pe:
                    if tn == "InstLdweights":
                        key = str(inst.ins[0])
                        if last == key and not (
                            inst.sync_info and inst.sync_info.on_wait
                        ):
                            ndel += 1
                            continue
                        last = key
                    elif tn not in ("InstMatmult", "InstNoOp"):
                        last = None
                new.append(inst)
            bb.instructions[:] = new
    return ndel


def _build_nc():
    nc = bass.Bass()
    xT = nc.declare_dram_parameter("xT", [C, NT], bf16, isOutput=False)
    # [p][oc: q01,k01,q23,k23][kc][m] -- per-partition lines contiguous
    wqk = nc.declare_dram_parameter("wqk", [128, 4, KC, 128], bf16, isOutput=False)
    wv = nc.declare_dram_parameter("wv", [128, KC, 256], bf16, isOutput=False)
    bqk = nc.declare_dram_parameter("bqk", [128, 4], f32, isOutput=False)
    bvr = nc.declare_dram_parameter("bvr", [128, 2, 2, 64], f32, isOutput=False)
    wrep = nc.declare_dram_parameter("wrep", [128, 2], f32, isOutput=False)
    woT = nc.declare_dram_parameter("woT", [2, 128, C], bf16, isOutput=False)
    y = nc.declare_dram_parameter("y", [NT, C], f32, isOutput=True)

    xT_r = xT.rearrange("(kc p) n -> kc p n", p=128)

    Exp = mybir.ActivationFunctionType.Exp
    Ln = mybir.ActivationFunctionType.Ln
    MUL = mybir.AluOpType.mult
    ADD = mybir.AluOpType.add

    with tile.TileContext(nc) as tc:
        with ExitStack() as ctx:
            perm = ctx.enter_context(tc.tile_pool(name="perm", bufs=1))

            w_sb = perm.tile([128, 4, KC, 128], bf16, name="w_sb", tag="w_sb")
            wv_sb = perm.tile([128, KC, 256], bf16, name="wv_sb", tag="wv_sb")
            bqk_sb = perm.tile([128, 5], f32, name="bqk_sb", tag="bqk_sb")
            bv_sb = perm.tile([128, 2, 2, 64], f32, name="bv_sb", tag="bv_sb")
            wrep_sb = perm.tile([128, 2], f32, name="wrep_sb", tag="wrep_sb")
            ones_mask = perm.tile([128, 128], bf16, name="ones_mask", tag="ones_mask")
            warm = perm.tile([128, 640], bf16, name="warm", tag="warm")
            xt_all = perm.tile([128, KC, NT], bf16, name="xt_all", tag="xt_all")
            woT_sb = [
                perm.tile([128, C], bf16, name=f"woT_sb{oo}", tag=f"woT_sb{oo}")
                for oo in range(2)
            ]
            qkpre = {}  # (qc, kind) -> pre-norm projection tile
            qhat = [
                perm.tile([128, NT], bf16, name=f"qhat{i}", tag=f"qhat{i}")
                for i in range(2)
            ]
            khat = [
                perm.tile([128, NT], bf16, name=f"khat{i}", tag=f"khat{i}")
                for i in range(2)
            ]
            # [jc][hb][o][d]: o=0 data, o=1 ones (denominator columns)
            vtok = [
                perm.tile([128, JC, 2, 2, 64], bf16, name=f"vtok{i}", tag=f"vtok{i}")
                for i in range(2)
            ]
            onT = [
                perm.tile([128, NT], bf16, name=f"onT{i}", tag=f"onT{i}")
                for i in range(2)
            ]

            qkpool = ctx.enter_context(tc.tile_pool(name="qkpool", bufs=2))
            sqpool = ctx.enter_context(tc.tile_pool(name="sqpool", bufs=2))
            t1pool = ctx.enter_context(tc.tile_pool(name="t1pool", bufs=1))
            rrpool = ctx.enter_context(tc.tile_pool(name="rrpool", bufs=2))
            ptpool = ctx.enter_context(tc.tile_pool(name="ptpool", bufs=6))
            dnpool = ctx.enter_context(tc.tile_pool(name="dnpool", bufs=2))
            ypool = ctx.enter_context(tc.tile_pool(name="ypool", bufs=2))
            spool = ctx.enter_context(tc.tile_pool(name="spool", bufs=2, space="PSUM"))
            pvpool = ctx.enter_context(
                tc.tile_pool(name="pvpool", bufs=1, space="PSUM")
            )

            cnt = [0]

            # ---------------- DMAs: 3 rings ----------------
            # vector ring: small tensors + memsets PE warmup needs
            nc.vector.memset(warm[:], 0.0)
            nc.scalar.dma_start(out=bqk_sb[:, 0:4], in_=bqk[:])
            nc.scalar.dma_start(out=bv_sb[:], in_=bvr[:])
            nc.scalar.dma_start(out=wrep_sb[:], in_=wrep[:])
            nc.vector.memset(bqk_sb[:, 4:5], EPS)
            nc.vector.memset(ones_mask[:], 0.0)
            nc.vector.memset(ones_mask[0:64, 0:64], 1.0)
            nc.vector.memset(ones_mask[64:128, 64:128], 1.0)
            # gpsimd ring: x chunks first, then vtok ones memsets
            for kc in range(KC - 2):
                eng = nc.gpsimd if kc % 2 == 0 else nc.scalar
                eng.dma_start(out=xt_all[:, kc], in_=xT_r[kc])
            for c in range(2):
                nc.gpsimd.memset(vtok[c][:, :, :, 1, :], 1.0)
            # sync ring: weights in first-use order
            nc.sync.dma_start(out=w_sb[:, 0], in_=wqk[:, 0])   # q01
            nc.sync.dma_start(out=w_sb[:, 1], in_=wqk[:, 1])   # k01
            nc.sync.dma_start(out=wv_sb[:], in_=wv[:])
            nc.sync.dma_start(out=w_sb[:, 2], in_=wqk[:, 2])   # q23
            nc.sync.dma_start(out=w_sb[:, 3], in_=wqk[:, 3])   # k23
            nc.sync.dma_start(out=xt_all[:, KC - 2], in_=xT_r[KC - 2])
            nc.sync.dma_start(out=xt_all[:, KC - 1], in_=xT_r[KC - 1])
            nc.sync.dma_start(out=woT_sb[0][:], in_=woT[0])
            nc.sync.dma_start(out=woT_sb[1][:], in_=woT[1])

            def emit_warm(n):
                cnt[0] += 1
                ps = spool.tile([128, 3 * FT], f32, name=f"warm{cnt[0]}", tag="s")
                for _ in range(n):
                    nc.tensor.matmul(
                        ps[:, 0:FT], lhsT=warm[:, 0:128], rhs=warm[:, 128:640],
                        start=True, stop=True,
                    )

            # ---------------- phase emitters ----------------
            def get_qk(qc, kind):
                key = (qc, kind)
                if key not in qkpre:
                    qkpre[key] = qkpool.tile(
                        [128, NT], f32, name=f"qk{qc}_{kind}", tag="qkpre"
                    )
                return qkpre[key]

            def emit_qkv_itile(qc, kind, ithalf, warm_after=False):
                """One i-tile (512 tokens) of q or k projection for head pair
                qc: 8 accumulating matmuls + bias add. Atomic (PSUM tile fully
                consumed at end) so it can be injected mid-attention."""
                oc = 2 * qc + kind
                isl = slice(ithalf * FT, (ithalf + 1) * FT)
                cnt[0] += 1
                ps = spool.tile([128, 3 * FT], f32, name=f"qp{cnt[0]}", tag="s")
                for kc in range(KC):
                    nc.tensor.matmul(
                        ps[:, 0:FT], lhsT=w_sb[:, oc, kc], rhs=xt_all[:, kc, isl],
                        start=(kc == 0), stop=(kc == KC - 1),
                    )
                    if warm_after and kc % 2 == 1 and kc < 7:
                        nc.tensor.matmul(
                            ps[:, FT : 2 * FT], lhsT=warm[:, 0:128],
                            rhs=warm[:, 128:640], start=True, stop=True,
                        )
                nc.vector.tensor_scalar_add(
                    get_qk(qc, kind)[:, isl], ps[:, 0:FT], bqk_sb[:, oc : oc + 1]
                )

            def make_rms_thunks(qc, kind):
                src = get_qk(qc, kind)
                dst = qhat[qc] if kind == 0 else khat[qc]
                st = {}

                def t_sq():
                    sq = sqpool.tile([128, NT], bf16, name=f"sq{qc}_{kind}", tag="sq")
                    nc.vector.tensor_mul(sq[:], src[:], src[:])
                    st["sq"] = sq
                    st["rr"] = rrpool.tile(
                        [128, NT], f32, name=f"rr{qc}_{kind}", tag="rr"
                    )

                def mk_half(half):
                    def t_half():
                        sq, rr = st["sq"], st["rr"]
                        hs = slice(half * 2 * FT, (half + 1) * 2 * FT)
                        cnt[0] += 1
                        ms = spool.tile(
                            [128, 3 * FT], f32, name=f"ms{cnt[0]}", tag="s"
                        )
                        for t in range(2):
                            tsl = slice(t * FT, (t + 1) * FT)
                            gsl = slice(
                                (half * 2 + t) * FT, (half * 2 + t + 1) * FT
                            )
                            nc.tensor.matmul(
                                ms[:, tsl], lhsT=ones_mask[:], rhs=sq[:, gsl],
                                start=True, stop=True,
                            )
                        t1 = t1pool.tile(
                            [128, 2 * FT], f32, name=f"t1_{qc}_{kind}_{half}", tag="t1"
                        )
                        nc.scalar.activation(
                            t1[:], ms[:, 0 : 2 * FT], Ln, scale=1.0 / D,
                            bias=bqk_sb[:, 4:5],
                        )
                        nc.scalar.activation(rr[:, hs], t1[:], Exp, scale=-0.5)
                    return t_half

                def t_app():
                    nc.vector.scalar_tensor_tensor(
                        dst[:], src[:], wrep_sb[:, kind : kind + 1], st["rr"][:],
                        MUL, MUL,
                    )

                return [t_sq, mk_half(0), mk_half(1), t_app]

            def emit_vT_chunk(c):
                """Token chunk c of V^T for all 4 heads: 8 accumulating
                matmuls (x-chunk stationaries) + 2 DVE bias-add drains into
                vtok (token-major, interleaved with ones columns)."""
                cnt[0] += 1
                vt = spool.tile(
                    [128, 12, 2, 64], f32, name=f"vt{cnt[0]}", tag="s"
                )
                for kc in range(KC):
                    nc.tensor.matmul(
                        vt[:, 0:2], lhsT=xt_all[:, kc, c * 128 : (c + 1) * 128],
                        rhs=wv_sb[:, kc], start=(kc == 0), stop=(kc == KC - 1),
                    )
                for qc in range(2):
                    nc.vector.tensor_tensor(
                        vtok[qc][:, c, :, 0, :], vt[:, qc], bv_sb[:, qc], ADD
                    )

            def emit_attention(qc, it, inject):
                """32 (head, jc) S-blocks, 3 per PSUM tile; PV skewed 2 tiles
                behind exp. After each 2-group of S+exp, pop one injected
                thunk so the PE queue always has independent work."""
                isl = slice(it * FT, (it + 1) * FT)
                pv2 = pvpool.tile([128, 2 * FT], f32, name=f"pv2_{qc}_{it}", tag="pv")
                NSEQ = 2 * JC
                PER = 3
                pending = []

                def emit_pv(pt3, seqs):
                    for sb in seqs:
                        hb = sb % 2
                        jb = sb // 2
                        bsl = slice((sb % PER) * FT, (sb % PER + 1) * FT)
                        nc.tensor.matmul(
                            pv2[:, hb * FT : (hb + 1) * FT],
                            lhsT=vtok[qc][:, jb, hb],
                            rhs=pt3[:, bsl],
                            start=(sb == hb),
                            stop=(sb >= NSEQ - 2),
                        )

                tiles = []
                sidx = 0
                while sidx < NSEQ:
                    n = min(PER, NSEQ - sidx)
                    tiles.append((sidx, n))
                    sidx += n

                def emit_s_tile(t0, n):
                    s3 = spool.tile(
                        [128, PER * FT], f32, name=f"s3_{qc}_{it}_{t0}", tag="s"
                    )
                    for k in range(n):
                        s = t0 + k
                        head = s % 2
                        jc = s // 2
                        nc.tensor.matmul(
                            s3[:, k * FT : (k + 1) * FT],
                            lhsT=khat[qc][head * 64 : (head + 1) * 64,
                                          jc * 128 : (jc + 1) * 128],
                            rhs=qhat[qc][head * 64 : (head + 1) * 64, isl],
                            start=True,
                            stop=True,
                        )
                    return s3

                def emit_exp(s3, t0, n):
                    pt3 = ptpool.tile(
                        [128, PER * FT], bf16, name=f"pt{qc}_{it}_{t0}", tag="pt"
                    )
                    nc.scalar.activation(
                        pt3[:, 0 : n * FT], s3[:, 0 : n * FT], Exp, scale=SCALE
                    )
                    pending.append((pt3, list(range(t0, t0 + n))))

                ti = 0
                while ti < len(tiles):
                    grp = tiles[ti : ti + 2]
                    ti += len(grp)
                    s3s = [emit_s_tile(t0, n) for t0, n in grp]
                    for s3g, (t0, n) in zip(s3s, grp):
                        emit_exp(s3g, t0, n)
                    while len(pending) > 2:
                        emit_pv(*pending.pop(0))
                    if inject:
                        inject.pop(0)()
                for args in pending:
                    emit_pv(*args)
                # normalize: O = PV * exp(-ln(denom)); denom replicated rows 64:128
                td = dnpool.tile([64, 2 * FT], f32, name=f"td{qc}_{it}", tag="td")
                nc.scalar.activation(td[:], pv2[64:128, :], Ln)
                bcr = dnpool.tile([64, 2 * FT], f32, name=f"bcr{qc}_{it}", tag="bcr")
                nc.scalar.activation(bcr[:], td[:], Exp, scale=-1.0)
                nc.vector.tensor_mul(
                    onT[qc][0:64, isl], pv2[0:64, 0:FT], bcr[:, 0:FT]
                )
                nc.vector.tensor_mul(
                    onT[qc][64:128, isl], pv2[0:64, FT : 2 * FT], bcr[:, FT : 2 * FT]
                )

            def emit_outproj(ic):
                csl = slice(ic * 128, (ic + 1) * 128)
                cnt[0] += 1
                p01 = spool.tile([128, 3 * FT], f32, name=f"p01_{ic}", tag="s")
                for oo in range(2):
                    nc.tensor.matmul(
                        p01[:, 0:FT],
                        lhsT=onT[oo][:, csl],
                        rhs=woT_sb[oo][:, 0:FT],
                        start=(oo == 0),
                        stop=(oo == 1),
                    )
                    nc.tensor.matmul(
                        p01[:, FT : 2 * FT],
                        lhsT=onT[oo][:, csl],
                        rhs=woT_sb[oo][:, FT : 2 * FT],
                        start=(oo == 0),
                        stop=(oo == 1),
                    )
                yt = ypool.tile([128, C], f32, name=f"yt{ic}", tag="yt")
                nc.vector.tensor_copy(yt[:], p01[:, 0 : 2 * FT])
                nc.sync.dma_start(out=y[csl, :], in_=yt[:])

            # ---------------- emission schedule ----------------
            emit_warm(8)
            # First i-tile of q AND k consumed kc-major (matches x DMA
            # arrival order); warm matmuls into unused PSUM regions of the
            # same tiles keep the PE p-state up through DMA pacing gaps.
            cnt[0] += 1
            psq = spool.tile([128, 3 * FT], f32, name=f"psq{cnt[0]}", tag="s")
            psk = spool.tile([128, 3 * FT], f32, name=f"psk{cnt[0]}", tag="s")
            for kc in range(KC):
                nc.tensor.matmul(
                    psq[:, 0:FT], lhsT=w_sb[:, 0, kc], rhs=xt_all[:, kc, 0:FT],
                    start=(kc == 0), stop=(kc == KC - 1),
                )
                nc.tensor.matmul(
                    psk[:, 0:FT], lhsT=w_sb[:, 1, kc], rhs=xt_all[:, kc, 0:FT],
                    start=(kc == 0), stop=(kc == KC - 1),
                )
                if kc < KC - 1:
                    for _ in range(2):
                        nc.tensor.matmul(
                            psq[:, FT : 2 * FT], lhsT=warm[:, 0:128],
                            rhs=warm[:, 128:640], start=True, stop=True,
                        )
            nc.vector.tensor_scalar_add(
                get_qk(0, 0)[:, 0:FT], psq[:, 0:FT], bqk_sb[:, 0:1]
            )
            nc.vector.tensor_scalar_add(
                get_qk(0, 1)[:, 0:FT], psk[:, 0:FT], bqk_sb[:, 1:2]
            )
            for ithalf in range(1, 4):
                emit_qkv_itile(0, 0, ithalf)
            for ithalf in range(1, 4):
                emit_qkv_itile(0, 1, ithalf)
            # V^T chunks with qc0 RMS thunks woven in (RMS waits on DVE
            # stats; vT keeps the PE busy meanwhile)
            rms0 = make_rms_thunks(0, 0) + make_rms_thunks(0, 1)
            for c in range(JC - 2):
                emit_vT_chunk(c)
                if c >= 1 and rms0:
                    rms0.pop(0)()
            while rms0:
                rms0.pop(0)()

            # attention for head pair 0, with the V^T tail and qc1 qkv/rms
            # injected
            inj = [
                (lambda c: lambda: emit_vT_chunk(c))(c)
                for c in range(JC - 2, JC)
            ]
            for kind in range(2):
                for ithalf in range(4):
                    inj.append(
                        (lambda k, ih: lambda: emit_qkv_itile(1, k, ih))(kind, ithalf)
                    )
                inj.extend(make_rms_thunks(1, kind))
            for it in range(TI):
                emit_attention(0, it, inj)
            while inj:
                inj.pop(0)()

            # attention for head pair 1, with out-projection of the previous
            # i-tile's tokens injected
            for it in range(TI):
                inj = (
                    []
                    if it == 0
                    else [
                        (lambda i: lambda: emit_outproj(i))(ic)
                        for ic in range((it - 1) * 4, it * 4)
                    ]
                )
                emit_attention(1, it, inj)
                while inj:
                    inj.pop(0)()
            for ic in range(12, 16):
                emit_outproj(ic)

    _split_waits(nc, limit=1)
    _dedupe_ldweights(nc)
    return nc


def _prep_inputs(x, Wq, bq, Wk, bk, Wv, bv, q_norm_w, k_norm_w, Wo, bo):
    bf = ml_dtypes.bfloat16
    x = np.asarray(x, dtype=np.float32)
    Wfull = np.concatenate(
        [np.asarray(Wq), np.asarray(Wk), np.asarray(Wv)], axis=0
    ).astype(np.float32)
    bfull = np.concatenate(
        [np.asarray(bq), np.asarray(bk), np.asarray(bv)], axis=0
    ).astype(np.float32)
    Wo = np.asarray(Wo, dtype=np.float32)
    q_norm_w = np.asarray(q_norm_w, dtype=np.float32)
    k_norm_w = np.asarray(k_norm_w, dtype=np.float32)

    xT_b = [np.ascontiguousarray(x[b].T).astype(bf) for b in range(B)]
    wrep = np.stack(
        [np.tile(q_norm_w, 2), np.tile(k_norm_w, 2)], axis=1
    ).astype(np.float32)

    in_maps = []
    for core in range(8):
        b = core // 4
        hg = core % 4
        heads = [hg * 4 + i for i in range(G)]
        qr = [Wfull[192 * h : 192 * h + 64] for h in heads]
        kr = [Wfull[192 * h + 64 : 192 * h + 128] for h in heads]
        vr = [Wfull[192 * h + 128 : 192 * h + 192] for h in heads]
        bqr = [bfull[192 * h : 192 * h + 64] for h in heads]
        bkr = [bfull[192 * h + 64 : 192 * h + 128] for h in heads]
        bvr_ = [bfull[192 * h + 128 : 192 * h + 192] for h in heads]

        # oc blocks: q01, k01, q23, k23  (each [128 out, 1024 in])
        blocks = [
            np.concatenate(qr[0:2], axis=0),
            np.concatenate(kr[0:2], axis=0),
            np.concatenate(qr[2:4], axis=0),
            np.concatenate(kr[2:4], axis=0),
        ]
        wqk_np = np.stack(blocks)  # [oc, m, in]
        wqk_np = np.ascontiguousarray(
            wqk_np.reshape(4, 128, KC, 128).transpose(3, 0, 2, 1)
        ).astype(bf)  # [p, oc, kc, m]
        bqk_np = np.stack(
            [
                np.concatenate(bqr[0:2]),
                np.concatenate(bkr[0:2]),
                np.concatenate(bqr[2:4]),
                np.concatenate(bkr[2:4]),
            ],
            axis=1,
        ).astype(np.float32)  # [128, 4]

        vrows = np.concatenate(vr, axis=0)  # [256 vch, 1024 in]
        wv_np = np.ascontiguousarray(
            vrows.reshape(256, KC, 128).transpose(2, 1, 0)
        ).astype(bf)  # [p, kc, vch]
        bv_np = np.broadcast_to(
            np.concatenate(bvr_).reshape(1, 2, 2, 64), (128, 2, 2, 64)
        ).astype(np.float32)

        cols = np.concatenate([np.arange(64 * h, 64 * h + 64) for h in heads])
        WoT_shard = np.ascontiguousarray(Wo[:, cols].T)  # [256, 1024]

        in_maps.append(
            {
                "xT": xT_b[b],
                "wqk": wqk_np,
                "wv": wv_np,
                "bqk": bqk_np,
                "bvr": np.ascontiguousarray(bv_np),
                "wrep": wrep,
                "woT": WoT_shard.reshape(2, 128, C).astype(bf),
            }
        )
    return in_maps


def kernel(**inputs):
    if "nc" not in _CACHE:
        _CACHE["nc"] = _build_nc()
    nc = _CACHE["nc"]
    in_maps = _prep_inputs(**inputs)
    res = run_bass_kernel_spmd(nc, in_maps, list(range(8)))
    bo = np.asarray(inputs["bo"], dtype=np.float32)
    y = np.zeros((B, N, C), dtype=np.float32)
    for core in range(8):
        y[core // 4] += res.results[core]["y"]
    y += bo[None, None, :]
    return y


# revision 19
# speedup vs baseline: 1.1434x; 1.1434x over previous
"""Trainium2 Bass kernel for nn_Attention_13537736917778.

Full inputs -> full output. Sharding: 8 cores = 2 (batch) x 4 (head groups of 4).
Per-core: channel-major flash attention (S^T layout, keys on partitions).
V is produced directly token-major (x-chunk stationaries) so no PE transposes
or PSUM->SBUF copy chains; softmax denominators come replicated from
ones-columns in the PV stationary and are inverted on DVE
(reciprocal_approx_fast), keeping the ACT engine exp-only during attention.
QKV/RMS for the second head pair and the out-projection are interleaved into
the attention emission in small atomic chunks so the PE queue never drains.
Out-projection partial sums are reduced on host.
"""
import sys
import numpy as np

sys.path.insert(0, "/opt/trn_rl_repo")

import ml_dtypes
import concourse.bass as bass
import concourse.mybir as mybir
from concourse import tile
from concourse.bass_utils import run_bass_kernel_spmd
from contextlib import ExitStack

bf16 = mybir.dt.bfloat16
f32 = mybir.dt.float32
f8 = mybir.dt.float8e4

B, N, C = 2, 2048, 1024
H, D = 16, 64
G = 4              # heads per core
NT = N             # tokens per core (one batch)
FT = 512
TI = NT // FT      # 4 i-tiles
KC = C // 128      # 8 input-channel chunks
JC = NT // 128     # 16 key chunks
EPS = 1e-6
SCALE = 1.0 / 8.0  # 1/sqrt(64)

_CACHE = {}


def _split_waits(nc, limit=1):
    """walrus CTRL has one hw wait slot; split multi-wait instructions into
    NOP chains carrying the extra waits."""
    counter = 0
    for fn in nc.m.functions:
        for bb in fn.blocks:
            new_insts = []
            for inst in bb.instructions:
                si = inst.sync_info
                if si is not None and si.on_wait and len(si.on_wait) > limit:
                    waits = list(si.on_wait)
                    head, tail = waits[:-limit], waits[-limit:]
                    for i in range(0, len(head), limit):
                        nop = mybir.InstNoOp(
                            name=f"I-waitsplit-{counter}", ins=[], outs=[]
                        )
                        counter += 1
                        nop.engine = inst.engine
                        nop.sync_info = mybir.SyncInfo(
                            on_wait=head[i : i + limit], on_update=[]
                        )
                        new_insts.append(nop)
                    inst.sync_info = mybir.SyncInfo(
                        on_wait=tail, on_update=list(si.on_update)
                    )
                new_insts.append(inst)
            bb.instructions[:] = new_insts
    return counter


def _dedupe_ldweights(nc):
    """Drop an InstLdweights identical to the previous one on the PE stream
    (only Matmult/NoOp between) -- the PE keeps the loaded weights."""
    pe = mybir.EngineType.PE
    ndel = 0
    for fn in nc.m.functions:
        for bb in fn.blocks:
            new = []
            last = None
            for inst in bb.instructions:
                tn = type(inst).__name__
                if getattr(inst, "engine", None) == pe:
                    if tn == "InstLdweights":
                        key = str(inst.ins[0])
                        if last == key and not (
                            inst.sync_info and inst.sync_info.on_wait
                        ):
                            ndel += 1
                            continue
                        last = key
                    elif tn not in ("InstMatmult", "InstNoOp"):
                        last = None
                new.append(inst)
            bb.instructions[:] = new
    return ndel


def _build_nc():
    nc = bass.Bass()
    xT = nc.declare_dram_parameter("xT", [C, NT], bf16, isOutput=False)
    # [p][oc: q01,k01,q23,k23][kc][m] -- per-partition lines contiguous
    wqk = nc.declare_dram_parameter("wqk", [128, 4, KC, 128], bf16, isOutput=False)
    wv = nc.declare_dram_parameter("wv", [128, KC, 256], bf16, isOutput=False)
    bqk = nc.declare_dram_parameter("bqk", [128, 4], f32, isOutput=False)
    bvr = nc.declare_dram_parameter("bvr", [128, 2, 2, 64], f32, isOutput=False)
    wrep = nc.declare_dram_parameter("wrep", [128, 2], f32, isOutput=False)
    woT = nc.declare_dram_parameter("woT", [2, 128, C], bf16, isOutput=False)
    y = nc.declare_dram_parameter("y", [NT, C], f32, isOutput=True)

    xT_r = xT.rearrange("(kc p) n -> kc p n", p=128)

    Exp = mybir.ActivationFunctionType.Exp
    Ln = mybir.ActivationFunctionType.Ln
    MUL = mybir.AluOpType.mult
    ADD = mybir.AluOpType.add

    with tile.TileContext(nc) as tc:
        with ExitStack() as ctx:
            perm = ctx.enter_context(tc.tile_pool(name="perm", bufs=1))

            w_sb = perm.tile([128, 4, KC, 128], bf16, name="w_sb", tag="w_sb")
            wv_sb = perm.tile([128, KC, 256], bf16, name="wv_sb", tag="wv_sb")
            bqk_sb = perm.tile([128, 5], f32, name="bqk_sb", tag="bqk_sb")
            bv_sb = perm.tile([128, 2, 2, 64], f32, name="bv_sb", tag="bv_sb")
            wrep_sb = perm.tile([128, 2], f32, name="wrep_sb", tag="wrep_sb")
            ones_mask = perm.tile([128, 128], bf16, name="ones_mask", tag="ones_mask")
            warm = perm.tile([128, 640], bf16, name="warm", tag="warm")
            xt_all = perm.tile([128, KC, NT], bf16, name="xt_all", tag="xt_all")
            woT_sb = [
                perm.tile([128, C], bf16, name=f"woT_sb{oo}", tag=f"woT_sb{oo}")
                for oo in range(2)
            ]
            qkpre = {}  # (qc, kind) -> pre-norm projection tile
            qhat = [
                perm.tile([128, NT], bf16, name=f"qhat{i}", tag=f"qhat{i}")
                for i in range(2)
            ]
            khat = [
                perm.tile([128, NT], bf16, name=f"khat{i}", tag=f"khat{i}")
                for i in range(2)
            ]
            # [jc][hb][o][d]: o=0 data, o=1 ones (denominator columns)
            vtok = [
                perm.tile([128, JC, 2, 2, 64], bf16, name=f"vtok{i}", tag=f"vtok{i}")
                for i in range(2)
            ]
            onT = [
                perm.tile([128, NT], bf16, name=f"onT{i}", tag=f"onT{i}")
                for i in range(2)
            ]

            qkpool = ctx.enter_context(tc.tile_pool(name="qkpool", bufs=2))
            sqpool = ctx.enter_context(tc.tile_pool(name="sqpool", bufs=2))
            t1pool = ctx.enter_context(tc.tile_pool(name="t1pool", bufs=1))
            rrpool = ctx.enter_context(tc.tile_pool(name="rrpool", bufs=2))
            ptpool = ctx.enter_context(tc.tile_pool(name="ptpool", bufs=6))
            dnpool = ctx.enter_context(tc.tile_pool(name="dnpool", bufs=2))
            ypool = ctx.enter_context(tc.tile_pool(name="ypool", bufs=2))
            spool = ctx.enter_context(tc.tile_pool(name="spool", bufs=2, space="PSUM"))
            pvpool = ctx.enter_context(
                tc.tile_pool(name="pvpool", bufs=1, space="PSUM")
            )

            cnt = [0]

            # ---------------- DMAs: 3 rings ----------------
            # vector ring: small tensors + memsets PE warmup needs
            nc.vector.memset(warm[:], 0.0)
            nc.scalar.dma_start(out=bqk_sb[:, 0:4], in_=bqk[:])
            nc.scalar.dma_start(out=bv_sb[:], in_=bvr[:])
            nc.scalar.dma_start(out=wrep_sb[:], in_=wrep[:])
            nc.vector.memset(bqk_sb[:, 4:5], EPS)
            nc.vector.memset(ones_mask[:], 0.0)
            nc.vector.memset(ones_mask[0:64, 0:64], 1.0)
            nc.vector.memset(ones_mask[64:128, 64:128], 1.0)
            # gpsimd ring: x chunks first, then vtok ones memsets
            for kc in range(KC):
                eng = nc.gpsimd if kc % 2 == 0 else nc.scalar
                eng.dma_start(out=xt_all[:, kc], in_=xT_r[kc])
            for c in range(2):
                nc.gpsimd.memset(vtok[c][:, :, :, 1, :], 1.0)
            # sync ring: weights in first-use order
            nc.sync.dma_start(out=w_sb[:, 0], in_=wqk[:, 0])   # q01
            nc.sync.dma_start(out=w_sb[:, 1], in_=wqk[:, 1])   # k01
            nc.sync.dma_start(out=wv_sb[:], in_=wv[:])
            nc.sync.dma_start(out=w_sb[:, 2], in_=wqk[:, 2])   # q23
            nc.sync.dma_start(out=w_sb[:, 3], in_=wqk[:, 3])   # k23
            nc.sync.dma_start(out=woT_sb[0][:], in_=woT[0])
            nc.sync.dma_start(out=woT_sb[1][:], in_=woT[1])

            def emit_warm(n):
                cnt[0] += 1
                ps = spool.tile([128, 3 * FT], f32, name=f"warm{cnt[0]}", tag="s")
                for _ in range(n):
                    nc.tensor.matmul(
                        ps[:, 0:FT], lhsT=warm[:, 0:128], rhs=warm[:, 128:640],
                        start=True, stop=True,
                    )

            # ---------------- phase emitters ----------------
            def get_qk(qc, kind):
                key = (qc, kind)
                if key not in qkpre:
                    qkpre[key] = qkpool.tile(
                        [128, NT], f32, name=f"qk{qc}_{kind}", tag="qkpre"
                    )
                return qkpre[key]

            def emit_qkv_itile(qc, kind, ithalf, warm_after=False):
                """One i-tile (512 tokens) of q or k projection for head pair
                qc: 8 accumulating matmuls + bias add. Atomic (PSUM tile fully
                consumed at end) so it can be injected mid-attention."""
                oc = 2 * qc + kind
                isl = slice(ithalf * FT, (ithalf + 1) * FT)
                cnt[0] += 1
                ps = spool.tile([128, 3 * FT], f32, name=f"qp{cnt[0]}", tag="s")
                for kc in range(KC):
                    nc.tensor.matmul(
                        ps[:, 0:FT], lhsT=w_sb[:, oc, kc], rhs=xt_all[:, kc, isl],
                        start=(kc == 0), stop=(kc == KC - 1),
                    )
                    if warm_after and kc % 2 == 1 and kc < 7:
                        nc.tensor.matmul(
                            ps[:, FT : 2 * FT], lhsT=warm[:, 0:128],
                            rhs=warm[:, 128:640], start=True, stop=True,
                        )
                nc.vector.tensor_scalar_add(
                    get_qk(qc, kind)[:, isl], ps[:, 0:FT], bqk_sb[:, oc : oc + 1]
                )

            def make_rms_thunks(qc, kind):
                src = get_qk(qc, kind)
                dst = qhat[qc] if kind == 0 else khat[qc]
                st = {}

                def t_sq():
                    sq = sqpool.tile([128, NT], bf16, name=f"sq{qc}_{kind}", tag="sq")
                    nc.vector.tensor_mul(sq[:], src[:], src[:])
                    st["sq"] = sq
                    st["rr"] = rrpool.tile(
                        [128, NT], f32, name=f"rr{qc}_{kind}", tag="rr"
                    )

                def mk_half(half):
                    def t_half():
                        sq, rr = st["sq"], st["rr"]
                        hs = slice(half * 2 * FT, (half + 1) * 2 * FT)
                        cnt[0] += 1
                        ms = spool.tile(
                            [128, 3 * FT], f32, name=f"ms{cnt[0]}", tag="s"
                        )
                        for t in range(2):
                            tsl = slice(t * FT, (t + 1) * FT)
                            gsl = slice(
                                (half * 2 + t) * FT, (half * 2 + t + 1) * FT
                            )
                            nc.tensor.matmul(
                                ms[:, tsl], lhsT=ones_mask[:], rhs=sq[:, gsl],
                                start=True, stop=True,
                            )
                        t1 = t1pool.tile(
                            [128, 2 * FT], f32, name=f"t1_{qc}_{kind}_{half}", tag="t1"
                        )
                        nc.scalar.activation(
                            t1[:], ms[:, 0 : 2 * FT], Ln, scale=1.0 / D,
                            bias=bqk_sb[:, 4:5],
                        )
                        nc.scalar.activation(rr[:, hs], t1[:], Exp, scale=-0.5)
                    return t_half

                def t_app():
                    nc.vector.scalar_tensor_tensor(
                        dst[:], src[:], wrep_sb[:, kind : kind + 1], st["rr"][:],
                        MUL, MUL,
                    )

                return [t_sq, mk_half(0), mk_half(1), t_app]

            def emit_vT_chunk(c):
                """Token chunk c of V^T for all 4 heads: 8 accumulating
                matmuls (x-chunk stationaries) + 2 DVE bias-add drains into
                vtok (token-major, interleaved with ones columns)."""
                cnt[0] += 1
                vt = spool.tile(
                    [128, 12, 2, 64], f32, name=f"vt{cnt[0]}", tag="s"
                )
                for kc in range(KC):
                    nc.tensor.matmul(
                        vt[:, 0:2], lhsT=xt_all[:, kc, c * 128 : (c + 1) * 128],
                        rhs=wv_sb[:, kc], start=(kc == 0), stop=(kc == KC - 1),
                    )
                for qc in range(2):
                    nc.vector.tensor_tensor(
                        vtok[qc][:, c, :, 0, :], vt[:, qc], bv_sb[:, qc], ADD
                    )

            def emit_attention(qc, it, inject):
                """32 (head, jc) S-blocks, 3 per PSUM tile; PV skewed 2 tiles
                behind exp. After each 2-group of S+exp, pop one injected
                thunk so the PE queue always has independent work."""
                isl = slice(it * FT, (it + 1) * FT)
                pv2 = pvpool.tile([128, 2 * FT], f32, name=f"pv2_{qc}_{it}", tag="pv")
                NSEQ = 2 * JC
                PER = 3
                pending = []

                def emit_pv(pt3, seqs):
                    for sb in seqs:
                        hb = sb % 2
                        jb = sb // 2
                        bsl = slice((sb % PER) * FT, (sb % PER + 1) * FT)
                        nc.tensor.matmul(
                            pv2[:, hb * FT : (hb + 1) * FT],
                            lhsT=vtok[qc][:, jb, hb],
                            rhs=pt3[:, bsl],
                            start=(sb == hb),
                            stop=(sb >= NSEQ - 2),
                        )

                tiles = []
                sidx = 0
                while sidx < NSEQ:
                    n = min(PER, NSEQ - sidx)
                    tiles.append((sidx, n))
                    sidx += n

                def emit_s_tile(t0, n):
                    s3 = spool.tile(
                        [128, PER * FT], f32, name=f"s3_{qc}_{it}_{t0}", tag="s"
                    )
                    for k in range(n):
                        s = t0 + k
                        head = s % 2
                        jc = s // 2
                        nc.tensor.matmul(
                            s3[:, k * FT : (k + 1) * FT],
                            lhsT=khat[qc][head * 64 : (head + 1) * 64,
                                          jc * 128 : (jc + 1) * 128],
                            rhs=qhat[qc][head * 64 : (head + 1) * 64, isl],
                            start=True,
                            stop=True,
                        )
                    return s3

                def emit_exp(s3, t0, n):
                    pt3 = ptpool.tile(
                        [128, PER * FT], bf16, name=f"pt{qc}_{it}_{t0}", tag="pt"
                    )
                    nc.scalar.activation(
                        pt3[:, 0 : n * FT], s3[:, 0 : n * FT], Exp, scale=SCALE
                    )
                    pending.append((pt3, list(range(t0, t0 + n))))

                ti = 0
                while ti < len(tiles):
                    grp = tiles[ti : ti + 2]
                    ti += len(grp)
                    s3s = [emit_s_tile(t0, n) for t0, n in grp]
                    for s3g, (t0, n) in zip(s3s, grp):
                        emit_exp(s3g, t0, n)
                    while len(pending) > 2:
                        emit_pv(*pending.pop(0))
                    if inject:
                        inject.pop(0)()
                for args in pending:
                    emit_pv(*args)
                # normalize: O = PV * exp(-ln(denom)); denom replicated rows 64:128
                td = dnpool.tile([64, 2 * FT], f32, name=f"td{qc}_{it}", tag="td")
                nc.scalar.activation(td[:], pv2[64:128, :], Ln)
                bcr = dnpool.tile([64, 2 * FT], f32, name=f"bcr{qc}_{it}", tag="bcr")
                nc.scalar.activation(bcr[:], td[:], Exp, scale=-1.0)
                nc.vector.tensor_mul(
                    onT[qc][0:64, isl], pv2[0:64, 0:FT], bcr[:, 0:FT]
                )
                nc.vector.tensor_mul(
                    onT[qc][64:128, isl], pv2[0:64, FT : 2 * FT], bcr[:, FT : 2 * FT]
                )

            def emit_outproj(ic):
                csl = slice(ic * 128, (ic + 1) * 128)
                cnt[0] += 1
                p01 = spool.tile([128, 3 * FT], f32, name=f"p01_{ic}", tag="s")
                for oo in range(2):
                    nc.tensor.matmul(
                        p01[:, 0:FT],
                        lhsT=onT[oo][:, csl],
                        rhs=woT_sb[oo][:, 0:FT],
                        start=(oo == 0),
                        stop=(oo == 1),
                    )
                    nc.tensor.matmul(
                        p01[:, FT : 2 * FT],
                        lhsT=onT[oo][:, csl],
                        rhs=woT_sb[oo][:, FT : 2 * FT],
                        start=(oo == 0),
                        stop=(oo == 1),
                    )
                yt = ypool.tile([128, C], f32, name=f"yt{ic}", tag="yt")
                nc.vector.tensor_copy(yt[:], p01[:, 0 : 2 * FT])
                nc.sync.dma_start(out=y[csl, :], in_=yt[:])

            # ---------------- emission schedule ----------------
            emit_warm(8)
            # First i-tile of q AND k consumed kc-major (matches x DMA
            # arrival order); warm matmuls into unused PSUM regions of the
            # same tiles keep the PE p-state up through DMA pacing gaps.
            cnt[0] += 1
            psq = spool.tile([128, 3 * FT], f32, name=f"psq{cnt[0]}", tag="s")
            psk = spool.tile([128, 3 * FT], f32, name=f"psk{cnt[0]}", tag="s")
            for kc in range(KC):
                nc.tensor.matmul(
                    psq[:, 0:FT], lhsT=w_sb[:, 0, kc], rhs=xt_all[:, kc, 0:FT],
                    start=(kc == 0), stop=(kc == KC - 1),
                )
                nc.tensor.matmul(
                    psk[:, 0:FT], lhsT=w_sb[:, 1, kc], rhs=xt_all[:, kc, 0:FT],
                    start=(kc == 0), stop=(kc == KC - 1),
                )
                if kc < KC - 1:
                    for _ in range(2):
                        nc.tensor.matmul(
                            psq[:, FT : 2 * FT], lhsT=warm[:, 0:128],
                            rhs=warm[:, 128:640], start=True, stop=True,
                        )
            nc.vector.tensor_scalar_add(
                get_qk(0, 0)[:, 0:FT], psq[:, 0:FT], bqk_sb[:, 0:1]
            )
            nc.vector.tensor_scalar_add(
                get_qk(0, 1)[:, 0:FT], psk[:, 0:FT], bqk_sb[:, 1:2]
            )
            for ithalf in range(1, 4):
                emit_qkv_itile(0, 0, ithalf)
            for ithalf in range(1, 4):
                emit_qkv_itile(0, 1, ithalf)
            # V^T chunks with qc0 RMS thunks woven in (RMS waits on DVE
            # stats; vT keeps the PE busy meanwhile)
            rms0 = make_rms_thunks(0, 0) + make_rms_thunks(0, 1)
            rms0_at = {1, 4, 6, 8, 10, 12, 14, 15}
            for c in range(JC):
                emit_vT_chunk(c)
                if c in rms0_at and rms0:
                    rms0.pop(0)()
            while rms0:
                rms0.pop(0)()

            # attention for head pair 0, with qc1 qkv/rms injected
            inj = []
            for kind in range(2):
                for ithalf in range(4):
                    inj.append(
                        (lambda k, ih: lambda: emit_qkv_itile(1, k, ih))(kind, ithalf)
                    )
                inj.extend(make_rms_thunks(1, kind))
            for it in range(TI):
                emit_attention(0, it, inj)
            while inj:
                inj.pop(0)()

            # attention for head pair 1, with out-projection of the previous
            # i-tile's tokens injected
            for it in range(TI):
                inj = (
                    []
                    if it == 0
                    else [
                        (lambda i: lambda: emit_outproj(i))(ic)
                        for ic in range((it - 1) * 4, it * 4)
                    ]
                )
                emit_attention(1, it, inj)
                while inj:
                    inj.pop(0)()
            for ic in range(12, 16):
                emit_outproj(ic)

    _split_waits(nc, limit=1)
    _dedupe_ldweights(nc)
    return nc


def _prep_inputs(x, Wq, bq, Wk, bk, Wv, bv, q_norm_w, k_norm_w, Wo, bo):
    bf = ml_dtypes.bfloat16
    x = np.asarray(x, dtype=np.float32)
    Wfull = np.concatenate(
        [np.asarray(Wq), np.asarray(Wk), np.asarray(Wv)], axis=0
    ).astype(np.float32)
    bfull = np.concatenate(
        [np.asarray(bq), np.asarray(bk), np.asarray(bv)], axis=0
    ).astype(np.float32)
    Wo = np.asarray(Wo, dtype=np.float32)
    q_norm_w = np.asarray(q_norm_w, dtype=np.float32)
    k_norm_w = np.asarray(k_norm_w, dtype=np.float32)

    xT_b = [np.ascontiguousarray(x[b].T).astype(bf) for b in range(B)]
    wrep = np.stack(
        [np.tile(q_norm_w, 2), np.tile(k_norm_w, 2)], axis=1
    ).astype(np.float32)

    in_maps = []
    for core in range(8):
        b = core // 4
        hg = core % 4
        heads = [hg * 4 + i for i in range(G)]
        qr = [Wfull[192 * h : 192 * h + 64] for h in heads]
        kr = [Wfull[192 * h + 64 : 192 * h + 128] for h in heads]
        vr = [Wfull[192 * h + 128 : 192 * h + 192] for h in heads]
        bqr = [bfull[192 * h : 192 * h + 64] for h in heads]
        bkr = [bfull[192 * h + 64 : 192 * h + 128] for h in heads]
        bvr_ = [bfull[192 * h + 128 : 192 * h + 192] for h in heads]

        # oc blocks: q01, k01, q23, k23  (each [128 out, 1024 in])
        blocks = [
            np.concatenate(qr[0:2], axis=0),
            np.concatenate(kr[0:2], axis=0),
            np.concatenate(qr[2:4], axis=0),
            np.concatenate(kr[2:4], axis=0),
        ]
        wqk_np = np.stack(blocks)  # [oc, m, in]
        wqk_np = np.ascontiguousarray(
            wqk_np.reshape(4, 128, KC, 128).transpose(3, 0, 2, 1)
        ).astype(bf)  # [p, oc, kc, m]
        bqk_np = np.stack(
            [
                np.concatenate(bqr[0:2]),
                np.concatenate(bkr[0:2]),
                np.concatenate(bqr[2:4]),
                np.concatenate(bkr[2:4]),
            ],
            axis=1,
        ).astype(np.float32)  # [128, 4]

        vrows = np.concatenate(vr, axis=0)  # [256 vch, 1024 in]
        wv_np = np.ascontiguousarray(
            vrows.reshape(256, KC, 128).transpose(2, 1, 0)
        ).astype(bf)  # [p, kc, vch]
        bv_np = np.broadcast_to(
            np.concatenate(bvr_).reshape(1, 2, 2, 64), (128, 2, 2, 64)
        ).astype(np.float32)

        cols = np.concatenate([np.arange(64 * h, 64 * h + 64) for h in heads])
        WoT_shard = np.ascontiguousarray(Wo[:, cols].T)  # [256, 1024]

        in_maps.append(
            {
                "xT": xT_b[b],
                "wqk": wqk_np,
                "wv": wv_np,
                "bqk": bqk_np,
                "bvr": np.ascontiguousarray(bv_np),
                "wrep": wrep,
                "woT": WoT_shard.reshape(2, 128, C).astype(bf),
            }
        )
    return in_maps


def kernel(**inputs):
    if "nc" not in _CACHE:
        _CACHE["nc"] = _build_nc()
    nc = _CACHE["nc"]
    in_maps = _prep_inputs(**inputs)
    res = run_bass_kernel_spmd(nc, in_maps, list(range(8)))
    bo = np.asarray(inputs["bo"], dtype=np.float32)
    y = np.zeros((B, N, C), dtype=np.float32)
    for core in range(8):
        y[core // 4] += res.results[core]["y"]
    y += bo[None, None, :]
    return y
